# revision 28
# baseline (speedup 1.0000x reference)
"""Trainium2 Bass kernel for nn_ContinousNormalizingFlowRHS.

Computes, for z in R^{B x Z} and scalar time t:
  h0 = tanh(W1*t + B1); h1 = tanh(einsum('knm,km->kn', W2, h0) + B2)
  w_in  = (W3_win  @ h1[0] + b3_win ).reshape(F, Z)
  w_out = (W3_wout @ h1[1] + b3_wout).reshape(F, Z)
  b     =  W3_b    @ h1[2] + b3_b
  gate  = sigmoid(W3_gate @ h1[3] + b3_gate)
  h = tanh(z @ w_in.T + b); dz = (h*gate) @ w_out / F
  trace = ((1-h^2)*gate) @ (sum(w_in*w_out,1)) / F
  out = concat([dz, -trace[:,None]], -1)

Strategy (8 NeuronCores, single SPMD launch):
  Phase A (F-sharded): each core streams its 1/8 of W3_win/W3_wout
  (bf16, [N, rows] transposed layout) on the two HWDGE queues and runs
  the matvec entirely on the PE as a stream of FWL stationary loads
  with a 1-column moving h1.  The psum naturally lands in w_inT
  ([z, f]) layout.  w_out is transposed on-chip, gate/bias folded, and
  the per-f trace weights sg are reduced locally.
  Handoff: two chunked AllGathers (one per local f-half) move the tiny
  per-core (w_inT, w_outg_fz, sg, b) slices to every core, overlapped
  with the second half of the stream.
  Phase B (B-sharded): each core computes only its own B/8 batch rows
  against the full F, accumulating dz/trace in PSUM across all 16
  f-blocks, then writes its output shard directly (no ReduceScatter).
"""

import sys
import types
import numpy as np
import ml_dtypes

BF = ml_dtypes.bfloat16

# problem sizes (hardcoded per contract)
Z = 128
N = 256
F = 2048
B = 8192
N_CORES = 8

FL = F // N_CORES          # local f per core (256)
NQ = 2                     # AllGather chunks (f-halves of 128)
FQ = FL // NQ              # f per chunk (128)
PE_F = 80                  # f per chunk computed on the PE (transposed layout)
DVE_F = FQ - PE_F          # f per chunk computed on the DVE (natural layout)
DCC = 8                    # f per DVE natural tile ([128, DCC*N])
CHUNK_R = 2048             # rows per streamed tile ([128, 2048] bf16)
BL = B // N_CORES          # batch shard (1024)
BC = 512                   # batch columns per stage-B psum chunk
CCW = 2 * FQ + 2           # cc payload cols: w_inT | w_outg_fz | sg | b


def _ensure_ntff_hook():
    """run_bass_kernel_spmd(trace=True) under axon needs antenv.axon_hooks."""
    if 'antenv.axon_hooks' in sys.modules:
        return
    try:
        from trn_agent_boot.trn_boot import _ntff_profile_via_ctypes
        hook = _ntff_profile_via_ctypes('/opt/axon/libaxon_pjrt.so')
    except Exception:
        hook = None
    try:
        import antenv
    except Exception:
        return
    mod = types.ModuleType('antenv.axon_hooks')
    mod.get_axon_ntff_profile_hook = lambda: hook
    mod.set_axon_ntff_profile_hook = lambda h: None
    sys.modules['antenv.axon_hooks'] = mod
    antenv.axon_hooks = mod


def build_module(n_cores=N_CORES, debug=False):
    import concourse.tile as tile
    from concourse import bacc, mybir

    F32 = mybir.dt.float32
    BF16 = mybir.dt.bfloat16
    ADD = mybir.AluOpType.add

    nrc = PE_F * Z // CHUNK_R    # PE tiles per (chunk, matrix, nb) (5)
    cpr = CHUNK_R // Z           # psum cols per streamed tile (16)
    ndt = DVE_F // DCC           # DVE natural tiles per (chunk, matrix) (6)

    nc = bacc.Bacc("TRN2", target_bir_lowering=False, debug=debug,
                   num_devices=n_cores)

    def inp(name, shape, dt):
        return nc.dram_tensor(name, shape, dt, kind="ExternalInput").ap()

    t_ap = inp("t", [1, 1], F32)
    w1_ap = inp("w1c", [128, 8], F32)
    b1_ap = inp("b1c", [128, 8], F32)
    b2_ap = inp("b2c", [128, 8], F32)
    w2t_ap = inp("w2tc", [128, 2048], BF16)
    w3T_aps = [[inp(f"w3{m}T_c{q}", [N, PE_F * Z], BF16) for q in range(NQ)]
               for m in ("win", "wout")]
    w3N_aps = [[inp(f"w3{m}N_c{q}", [ndt * 128, DCC * N], BF16)
                for q in range(NQ)] for m in ("win", "wout")]
    ones_ap = inp("onesb", [1, 128], BF16)
    b3winT_ap = inp("b3winT_c", [128, FL], F32)
    b3woutT_ap = inp("b3woutT_c", [128, FL], F32)
    w3bT_ap = inp("w3bT_c", [N, FL], BF16)
    w3gateT_ap = inp("w3gateT_c", [N, FL], BF16)
    b3b_ap = inp("b3b_c", [128, NQ], F32)
    b3gate_ap = inp("b3gate_c", [128, NQ], F32)
    zt_ap = inp("ztk", [128, BL], BF16)
    eye_ap = inp("eyeb", [128, 128], BF16)
    out_ap = nc.dram_tensor("out", [Z + 1, BL], F32,
                            kind="ExternalOutput").ap()

    with tile.TileContext(nc) as tc:
        with tc.tile_pool(name="persist", bufs=1) as pp, \
             tc.tile_pool(name="strm_sy", bufs=8) as sp_sy, \
             tc.tile_pool(name="strm_sc", bufs=8) as sp_sc, \
             tc.tile_pool(name="strm_gp", bufs=4) as sp_gp, \
             tc.tile_pool(name="work", bufs=3) as wp, \
             tc.tile_pool(name="ccsb", bufs=2) as cp, \
             tc.tile_pool(name="ps_misc", bufs=1, space="PSUM") as ps_misc, \
             tc.tile_pool(name="ps_mv", bufs=1, space="PSUM") as ps_mv, \
             tc.tile_pool(name="ps_h", bufs=2, space="PSUM") as ps_h, \
             tc.tile_pool(name="ps_dz", bufs=1, space="PSUM") as ps_dz, \
             tc.tile_pool(name="ps_tr", bufs=1, space="PSUM") as ps_tr, \
             tc.tile_pool(name="dram", bufs=1, space="DRAM") as dp:

            # ---- small loads (SWDGE queue; issue before the big stream) --
            t_bc = pp.tile([128, 1], F32, tag="tbc")
            nc.gpsimd.dma_start(t_bc[:], t_ap.broadcast_to([128, 1]))
            w1_sb = pp.tile([128, 8], F32, tag="w1")
            b1_sb = pp.tile([128, 8], F32, tag="b1")
            b2_sb = pp.tile([128, 8], F32, tag="b2")
            w2t_sb = pp.tile([128, 2048], BF16, tag="w2t")
            nc.gpsimd.dma_start(w1_sb[:], w1_ap[:])
            nc.gpsimd.dma_start(b1_sb[:], b1_ap[:])
            nc.gpsimd.dma_start(b2_sb[:], b2_ap[:])
            nc.gpsimd.dma_start(w2t_sb[:], w2t_ap[:])
            zt_sb = pp.tile([128, BL], BF16, tag="zt")
            nc.gpsimd.dma_start(zt_sb[:], zt_ap[:])
            eye_sb = pp.tile([128, 128], BF16, tag="eye")
            nc.gpsimd.dma_start(eye_sb[:], eye_ap[:])
            b3winT_sb = pp.tile([128, FL], F32, tag="b3winT")
            b3woutT_sb = pp.tile([128, FL], F32, tag="b3woutT")
            nc.gpsimd.dma_start(b3winT_sb[:], b3winT_ap[:])
            nc.gpsimd.dma_start(b3woutT_sb[:], b3woutT_ap[:])
            w3h_sb = []
            for m, ap in (("b", w3bT_ap), ("gate", w3gateT_ap)):
                hb = pp.tile([128, 2 * FL], BF16, tag=f"w3{m}T")
                nc.gpsimd.dma_start(
                    hb[:], ap.rearrange("(nb p) fl -> p nb fl", p=128))
                w3h_sb.append(hb)
            b3b_sb = pp.tile([128, NQ], F32, tag="b3b")
            b3gate_sb = pp.tile([128, NQ], F32, tag="b3gate")
            nc.gpsimd.dma_start(b3b_sb[:], b3b_ap[:])
            nc.gpsimd.dma_start(b3gate_sb[:], b3gate_ap[:])

            # ---- parameter nets (tiny) ----------------------------------
            h0pre = pp.tile([128, 8], F32, tag="h0pre")
            nc.vector.tensor_scalar_mul(h0pre[:], w1_sb[:], t_bc[:, 0:1])
            nc.vector.tensor_add(h0pre[:], h0pre[:], b1_sb[:])
            h0_sb = pp.tile([128, 8], BF16, tag="h0")
            nc.scalar.activation(h0_sb[:], h0pre[:],
                                 mybir.ActivationFunctionType.Tanh)
            ps_h1 = ps_misc.tile([128, 8], F32, tag="misc")
            for k4 in range(4):
                for nb in range(2):
                    c = k4 * 2 + nb
                    for mb in range(2):
                        lhs = w2t_sb[:, k4 * 512 + mb * 256 + nb * 128:
                                     k4 * 512 + mb * 256 + nb * 128 + 128]
                        nc.tensor.matmul(ps_h1[:, c:c + 1], lhs,
                                         h0_sb[:, k4 * 2 + mb:k4 * 2 + mb + 1],
                                         start=(mb == 0), stop=(mb == 1))
            h1pre = pp.tile([128, 8], F32, tag="h1pre")
            h1_sb = pp.tile([128, 8], BF16, tag="h1")
            nc.vector.tensor_add(h1pre[:], ps_h1[:], b2_sb[:])
            nc.scalar.activation(h1_sb[:], h1pre[:],
                                 mybir.ActivationFunctionType.Tanh)

            # ---- h1 replicated across partitions for the DVE matvec ----
            ones_sb = pp.tile([1, 128], BF16, tag="ones")
            nc.gpsimd.dma_start(ones_sb[:], ones_ap[:])
            h1rep = []
            for net in range(2):
                h1row = pp.tile([1, N], BF16, tag=f"h1row{net}")
                for nb in range(2):
                    h1T = ps_misc.tile([1, 128], BF16, tag="misc",
                                       name="h1T")
                    nc.tensor.transpose(
                        h1T[:], h1_sb[:, net * 2 + nb:net * 2 + nb + 1],
                        eye_sb[:])
                    nc.scalar.activation(
                        h1row[0:1, nb * 128:(nb + 1) * 128],
                        h1T[:], mybir.ActivationFunctionType.Identity)
                ps_rep = ps_misc.tile([128, N], F32, tag="misc", name="psrep")
                nc.tensor.matmul(ps_rep[:], ones_sb[:], h1row[:],
                                 start=True, stop=True)
                hr = pp.tile([128, N], BF16, tag=f"h1rep{net}")
                nc.vector.tensor_copy(hr[:], ps_rep[:])
                h1rep.append(hr)

            # ---- heads: b and gate (local f, [128 f, 2] psum cols) ------
            b_loc = pp.tile([128, NQ], F32, tag="bloc")
            gate_loc = pp.tile([128, NQ], F32, tag="gateloc")
            gpre = pp.tile([128, NQ], F32, tag="gpre")
            for hb, dst, net in ((w3h_sb[0], b_loc, 2), (w3h_sb[1], gpre, 3)):
                ph = ps_misc.tile([128, NQ], F32, tag="misc", name="phd")
                for j in range(NQ):
                    for nb in range(2):
                        nc.tensor.matmul(
                            ph[:, j:j + 1],
                            hb[:, nb * FL + j * 128:nb * FL + (j + 1) * 128],
                            h1_sb[:, net * 2 + nb:net * 2 + nb + 1],
                            start=(nb == 0), stop=(nb == 1))
                bias = b3b_sb if net == 2 else b3gate_sb
                nc.vector.tensor_add(dst[:], ph[:], bias[:])
            nc.scalar.activation(gate_loc[:], gpre[:],
                                 mybir.ActivationFunctionType.Sigmoid)

            # ---- phase A: PE matvec + per-chunk AllGather ---------------
            cc_in = [dp.tile([128, CCW], BF16, tag=f"ccin{q}",
                             name=f"ccin{q}") for q in range(NQ)]
            cc_out = [dp.tile([n_cores, 128, CCW], BF16, tag=f"ccout{q}",
                              name=f"ccout{q}") for q in range(NQ)]
            ag_sb = []
            sg_f32 = pp.tile([128, NQ], F32, tag="sg")
            # engine stripe: 2x sync, 2x scalar, 1x gpsimd per 5 tiles —
            # ring pacing on each queue then tracks consumption order.
            stripe = [(nc.sync, sp_sy), (nc.scalar, sp_sc),
                      (nc.sync, sp_sy), (nc.scalar, sp_sc),
                      (nc.gpsimd, sp_gp)]
            for q in range(NQ):
                # chunk's stream DMA issue, in consumption order
                streamT = {}
                streamN = {}
                ti = 0
                for m in range(2):
                    for rc in range(nrc):
                        for nb in range(2):
                            eng, pool = stripe[ti % 5]
                            ti += 1
                            w3t = pool.tile([128, CHUNK_R], BF16, tag="w3s")
                            eng.dma_start(
                                w3t[:],
                                w3T_aps[m][q][nb * 128:(nb + 1) * 128,
                                              rc * CHUNK_R:(rc + 1) * CHUNK_R])
                            streamT[(m, rc, nb)] = w3t
                    for t in range(ndt):
                        eng, pool = stripe[ti % 5]
                        ti += 1
                        w3n = pool.tile([128, DCC * N], BF16, tag="w3s")
                        eng.dma_start(
                            w3n[:], w3N_aps[m][q][t * 128:(t + 1) * 128, :])
                        streamN[(m, t)] = w3n
                mv = []
                acc = []
                for m, net in ((0, 0), (1, 1)):
                    pw = ps_mv.tile([128, PE_F], F32, tag="mv", name=f"mv{m}")
                    for rc in range(nrc):
                        for j in range(cpr):
                            col = rc * cpr + j
                            for nb in range(2):
                                w3t = streamT[(m, rc, nb)]
                                nc.tensor.matmul(
                                    pw[:, col:col + 1],
                                    w3t[:, j * 128:(j + 1) * 128],
                                    h1_sb[:, net * 2 + nb:net * 2 + nb + 1],
                                    start=(nb == 0), stop=(nb == 1))
                    mv.append(pw)
                    # DVE part: natural-layout multiply by replicated h1,
                    # then reduce over n.
                    ad = wp.tile([128, DVE_F], F32, tag=f"acc{m}")
                    hr_bc = h1rep[net][:].unsqueeze(1).broadcast_to(
                        [128, DCC, N])
                    for t in range(ndt):
                        w3n = streamN[(m, t)]
                        prod = wp.tile([128, DCC * N], BF16, tag="prod")
                        nc.vector.tensor_mul(
                            prod[:].rearrange("p (c n) -> p c n", n=N),
                            w3n[:].rearrange("p (c n) -> p c n", n=N), hr_bc)
                        nc.vector.tensor_reduce(
                            ad[:, t * DCC:(t + 1) * DCC],
                            prod[:].rearrange("p (c n) -> p c n", n=N),
                            mybir.AxisListType.X, ADD)
                    acc.append(ad)
                cc_sb = cp.tile([128, CCW], BF16, tag="ccsb")
                # w_inT (+bias) -> cc cols [0, FQ)
                nc.vector.tensor_add(cc_sb[:, 0:PE_F], mv[0][:],
                                     b3winT_sb[:, q * FQ:q * FQ + PE_F])
                nc.vector.tensor_add(cc_sb[:, PE_F:FQ], acc[0][:],
                                     b3winT_sb[:, q * FQ + PE_F:(q + 1) * FQ])
                woutTb = wp.tile([128, FQ], BF16, tag="woutTb")
                nc.vector.tensor_add(woutTb[:, 0:PE_F], mv[1][:],
                                     b3woutT_sb[:, q * FQ:q * FQ + PE_F])
                nc.vector.tensor_add(woutTb[:, PE_F:FQ], acc[1][:],
                                     b3woutT_sb[:, q * FQ + PE_F:(q + 1) * FQ])
                tpsB = ps_misc.tile([128, 128], BF16, tag="misc", name="tpsB")
                nc.tensor.transpose(tpsB[:], woutTb[:], eye_sb[:])
                # w_outg_fz = w_out_fz * gate -> cc cols [FQ, 2FQ)
                nc.vector.tensor_scalar_mul(cc_sb[:, FQ:2 * FQ], tpsB[:],
                                            gate_loc[:, q:q + 1])
                tpsA = ps_misc.tile([128, 128], BF16, tag="misc", name="tpsA")
                nc.tensor.transpose(tpsA[:], cc_sb[:, 0:FQ], eye_sb[:])
                prod = wp.tile([128, 128], F32, tag="sgprod")
                nc.vector.tensor_mul(prod[:], tpsA[:], cc_sb[:, FQ:2 * FQ])
                nc.vector.tensor_reduce(sg_f32[:, q:q + 1], prod[:],
                                        mybir.AxisListType.X, ADD)
                nc.vector.tensor_copy(cc_sb[:, 2 * FQ:2 * FQ + 1],
                                      sg_f32[:, q:q + 1])
                nc.vector.tensor_copy(cc_sb[:, 2 * FQ + 1:2 * FQ + 2],
                                      b_loc[:, q:q + 1])
                nc.gpsimd.dma_start(cc_in[q][:], cc_sb[:])
                nc.gpsimd.collective_compute(
                    "AllGather", mybir.AluOpType.bypass,
                    replica_groups=[list(range(n_cores))],
                    ins=[cc_in[q].opt()], outs=[cc_out[q].opt()])
                ag = pp.tile([128, n_cores * CCW], BF16, tag=f"ag{q}")
                nc.gpsimd.dma_start(
                    ag[:], cc_out[q].rearrange("k p c -> p k c"))
                ag_sb.append(ag)

            # ---- phase B: B-sharded batch matmuls over full F -----------
            # f-blocks ordered q-major so all of AG chunk 0 is consumed
            # before anything waits on AG chunk 1.
            nj = BL // BC
            nfb = F // 128
            b_f32 = pp.tile([128, nfb], F32, tag="bf32")
            pdz = [ps_dz.tile([128, BC], F32, tag=f"pdz{j}", name=f"pdz{j}")
                   for j in range(nj)]
            ptr = [ps_tr.tile([1, BC], F32, tag=f"ptr{j}", name=f"ptr{j}")
                   for j in range(nj)]
            ag3d = [ag_sb[q].rearrange("p (k c) -> p k c", c=CCW)
                    for q in range(NQ)]
            for q in range(NQ):
                nc.vector.tensor_copy(
                    b_f32[:, q * n_cores:(q + 1) * n_cores],
                    ag3d[q][:, :, 2 * FQ + 1])
                for kk in range(n_cores):
                    i = q * n_cores + kk
                    ag = ag_sb[q]
                    lhT = ag[:, kk * CCW:kk * CCW + FQ]
                    lhD = ag[:, kk * CCW + FQ:kk * CCW + 2 * FQ]
                    sgc = ag[:, kk * CCW + 2 * FQ:kk * CCW + 2 * FQ + 1]
                    for j in range(nj):
                        ph = ps_h.tile([128, BC], F32, tag="ph")
                        nc.tensor.matmul(ph[:], lhT,
                                         zt_sb[:, j * BC:(j + 1) * BC],
                                         start=True, stop=True)
                        h_bf = wp.tile([128, BC], BF16, tag="hbf")
                        nc.scalar.activation(h_bf[:], ph[:],
                                             mybir.ActivationFunctionType.Tanh,
                                             bias=b_f32[:, i:i + 1])
                        h2_bf = wp.tile([128, BC], BF16, tag="h2bf")
                        nc.vector.tensor_mul(h2_bf[:], h_bf[:], h_bf[:])
                        nc.tensor.matmul(pdz[j][:], lhD, h_bf[:],
                                         start=(i == 0), stop=(i == nfb - 1))
                        nc.tensor.matmul(ptr[j][:], sgc, h2_bf[:],
                                         start=(i == 0), stop=(i == nfb - 1))
            # trace constant: cneg = -sum_f sg / F (issued after all h2
            # muls so the DVE FIFO never blocks on the second AllGather)
            s1 = pp.tile([128, NQ], F32, tag="s1")
            for q in range(NQ):
                nc.vector.tensor_reduce(s1[:, q:q + 1],
                                        ag3d[q][:, :, 2 * FQ],
                                        mybir.AxisListType.X, ADD)
            s1t = pp.tile([128, 1], F32, tag="s1t")
            nc.vector.tensor_reduce(s1t[:], s1[:], mybir.AxisListType.X, ADD)
            s128 = pp.tile([128, 1], F32, tag="s128")
            from concourse import bass_isa
            nc.gpsimd.partition_all_reduce(s128[:], s1t[:], 128,
                                           bass_isa.ReduceOp.add)
            cneg = pp.tile([1, 1], F32, tag="cneg")
            nc.scalar.mul(cneg[:], s128[0:1, 0:1], -1.0 / F)
            for j in range(nj):
                dz_sb = wp.tile([128, BC], F32, tag="dzsb")
                nc.scalar.mul(dz_sb[:], pdz[j][:], 1.0 / F)
                nc.sync.dma_start(out_ap[0:Z, j * BC:(j + 1) * BC], dz_sb[:])
                tr_sb = wp.tile([1, BC], F32, tag="trsb")
                nc.scalar.activation(tr_sb[:], ptr[j][:],
                                     mybir.ActivationFunctionType.Identity,
                                     bias=cneg[0:1, 0:1], scale=1.0 / F)
                nc.sync.dma_start(out_ap[Z:Z + 1, j * BC:(j + 1) * BC],
                                  tr_sb[:])

    nc.compile()
    return nc


def host_prep(t, z_and_logpz, W1, B1, W2, B2, W3_win, b3_win,
              W3_wout, b3_wout, W3_b, b3_b, W3_gate, b3_gate,
              n_cores=N_CORES):
    """Shard + lay out the numpy inputs into per-core in_maps."""
    def col8(x):  # [4, 256] -> [128, 8] with col = k*2 + nb
        return np.ascontiguousarray(
            np.asarray(x, np.float32).reshape(4, 2, 128).transpose(2, 0, 1)
            .reshape(128, 8))

    t_in = np.asarray(t, np.float32).reshape(1, 1)
    w1c = col8(np.asarray(W1, np.float32)[:, :, 0])
    b1c = col8(B1)
    b2c = col8(B2)
    w2tc = np.ascontiguousarray(
        np.asarray(W2, np.float32).transpose(0, 2, 1)
        .reshape(4, 2, 128, 256).transpose(2, 0, 1, 3).reshape(128, 2048)
    ).astype(BF)
    w3win_bf = np.asarray(W3_win, np.float32).astype(BF)
    w3wout_bf = np.asarray(W3_wout, np.float32).astype(BF)
    w3b_bf = np.asarray(W3_b, np.float32).astype(BF)
    w3gate_bf = np.asarray(W3_gate, np.float32).astype(BF)
    b3win = np.asarray(b3_win, np.float32)
    b3wout = np.asarray(b3_wout, np.float32)
    z = np.asarray(z_and_logpz, np.float32)[:, :Z]
    ztb = np.ascontiguousarray(z.T).astype(BF)   # [Z, B]
    eye = np.eye(128, dtype=np.float32).astype(BF)

    def pack_nat(x):  # [DVE_F*Z, N] -> [ndt*128, DCC*N], partition-contig
        nt = x.shape[0] // (DCC * 128)
        return np.ascontiguousarray(
            x.reshape(nt, DCC, 128, N).transpose(0, 2, 1, 3)
            .reshape(nt * 128, DCC * N))

    ones = np.ones((1, 128), dtype=np.float32).astype(BF)
    rows = FL * Z            # per-core W3 rows (32768)
    in_maps = []
    for k in range(n_cores):
        r0 = k * rows
        f0 = k * FL
        im = {
            "t": t_in, "w1c": w1c, "b1c": b1c, "b2c": b2c, "w2tc": w2tc,
            "onesb": ones,
            "b3winT_c": np.ascontiguousarray(
                b3win[r0:r0 + rows].reshape(FL, Z).T),
            "b3woutT_c": np.ascontiguousarray(
                b3wout[r0:r0 + rows].reshape(FL, Z).T),
            "w3bT_c": np.ascontiguousarray(w3b_bf[f0:f0 + FL].T),
            "w3gateT_c": np.ascontiguousarray(w3gate_bf[f0:f0 + FL].T),
            "b3b_c": np.ascontiguousarray(
                np.asarray(b3_b, np.float32)[f0:f0 + FL].reshape(NQ, 128).T),
            "b3gate_c": np.ascontiguousarray(
                np.asarray(b3_gate, np.float32)[f0:f0 + FL].reshape(NQ, 128).T),
            "ztk": np.ascontiguousarray(ztb[:, k * BL:(k + 1) * BL]),
            "eyeb": eye,
        }
        for q in range(NQ):
            rq0 = r0 + q * FQ * Z
            rpe = PE_F * Z
            rdv = DVE_F * Z
            im[f"w3winT_c{q}"] = np.ascontiguousarray(
                w3win_bf[rq0:rq0 + rpe].T)
            im[f"w3woutT_c{q}"] = np.ascontiguousarray(
                w3wout_bf[rq0:rq0 + rpe].T)
            im[f"w3winN_c{q}"] = pack_nat(w3win_bf[rq0 + rpe:rq0 + rpe + rdv])
            im[f"w3woutN_c{q}"] = pack_nat(
                w3wout_bf[rq0 + rpe:rq0 + rpe + rdv])
        in_maps.append(im)
    return in_maps


_NC_CACHE = {}


def kernel(**inputs) -> np.ndarray:
    _ensure_ntff_hook()
    from concourse import bass_utils

    key = "full"
    if key not in _NC_CACHE:
        _NC_CACHE[key] = build_module()
    nc = _NC_CACHE[key]

    in_maps = host_prep(**inputs)
    res = bass_utils.run_bass_kernel_spmd(nc, in_maps, list(range(N_CORES)))
    out = np.empty((B, Z + 1), np.float32)
    for k in range(N_CORES):
        out[k * BL:(k + 1) * BL, :] = res.results[k]["out"].T
    return out


# revision 29
# speedup vs baseline: 1.0478x; 1.0478x over previous
"""Trainium2 Bass kernel for nn_ContinousNormalizingFlowRHS.

Computes, for z in R^{B x Z} and scalar time t:
  h0 = tanh(W1*t + B1); h1 = tanh(einsum('knm,km->kn', W2, h0) + B2)
  w_in  = (W3_win  @ h1[0] + b3_win ).reshape(F, Z)
  w_out = (W3_wout @ h1[1] + b3_wout).reshape(F, Z)
  b     =  W3_b    @ h1[2] + b3_b
  gate  = sigmoid(W3_gate @ h1[3] + b3_gate)
  h = tanh(z @ w_in.T + b); dz = (h*gate) @ w_out / F
  trace = ((1-h^2)*gate) @ (sum(w_in*w_out,1)) / F
  out = concat([dz, -trace[:,None]], -1)

Strategy (8 NeuronCores, single SPMD launch):
  Phase A (F-sharded): each core streams its 1/8 of W3_win/W3_wout
  (bf16, [N, rows] transposed layout) on the two HWDGE queues and runs
  the matvec entirely on the PE as a stream of FWL stationary loads
  with a 1-column moving h1.  The psum naturally lands in w_inT
  ([z, f]) layout.  w_out is transposed on-chip, gate/bias folded, and
  the per-f trace weights sg are reduced locally.
  Handoff: two chunked AllGathers (one per local f-half) move the tiny
  per-core (w_inT, w_outg_fz, sg, b) slices to every core, overlapped
  with the second half of the stream.
  Phase B (B-sharded): each core computes only its own B/8 batch rows
  against the full F, accumulating dz/trace in PSUM across all 16
  f-blocks, then writes its output shard directly (no ReduceScatter).
"""

import sys
import types
import numpy as np
import ml_dtypes

BF = ml_dtypes.bfloat16

# problem sizes (hardcoded per contract)
Z = 128
N = 256
F = 2048
B = 8192
N_CORES = 8

FL = F // N_CORES          # local f per core (256)
NQ = 2                     # AllGather chunks (f-halves of 128)
FQ = FL // NQ              # f per chunk (128)
PE_F = 96                  # f per chunk computed on the PE (transposed layout)
DVE_F = FQ - PE_F          # f per chunk computed on the DVE (natural layout)
DCC = 8                    # f per DVE natural tile ([128, DCC*N])
CHUNK_R = 2048             # rows per streamed tile ([128, 2048] bf16)
BL = B // N_CORES          # batch shard (1024)
BC = 512                   # batch columns per stage-B psum chunk
CCW = 2 * FQ + 2           # cc payload cols: w_inT | w_outg_fz | sg | b


def _ensure_ntff_hook():
    """run_bass_kernel_spmd(trace=True) under axon needs antenv.axon_hooks."""
    if 'antenv.axon_hooks' in sys.modules:
        return
    try:
        from trn_agent_boot.trn_boot import _ntff_profile_via_ctypes
        hook = _ntff_profile_via_ctypes('/opt/axon/libaxon_pjrt.so')
    except Exception:
        hook = None
    try:
        import antenv
    except Exception:
        return
    mod = types.ModuleType('antenv.axon_hooks')
    mod.get_axon_ntff_profile_hook = lambda: hook
    mod.set_axon_ntff_profile_hook = lambda h: None
    sys.modules['antenv.axon_hooks'] = mod
    antenv.axon_hooks = mod


def build_module(n_cores=N_CORES, debug=False):
    import concourse.tile as tile
    from concourse import bacc, mybir

    F32 = mybir.dt.float32
    BF16 = mybir.dt.bfloat16
    ADD = mybir.AluOpType.add

    nrc = PE_F * Z // CHUNK_R    # PE tiles per (chunk, matrix, nb) (5)
    cpr = CHUNK_R // Z           # psum cols per streamed tile (16)
    ndt = DVE_F // DCC           # DVE natural tiles per (chunk, matrix) (6)

    nc = bacc.Bacc("TRN2", target_bir_lowering=False, debug=debug,
                   num_devices=n_cores)

    def inp(name, shape, dt):
        return nc.dram_tensor(name, shape, dt, kind="ExternalInput").ap()

    t_ap = inp("t", [1, 1], F32)
    w1_ap = inp("w1c", [128, 8], F32)
    b1_ap = inp("b1c", [128, 8], F32)
    b2_ap = inp("b2c", [128, 8], F32)
    w2t_ap = inp("w2tc", [128, 2048], BF16)
    w3T_aps = [[inp(f"w3{m}T_c{q}", [N, PE_F * Z], BF16) for q in range(NQ)]
               for m in ("win", "wout")]
    w3N_aps = [[inp(f"w3{m}N_c{q}", [ndt * 128, DCC * N], BF16)
                for q in range(NQ)] for m in ("win", "wout")]
    ones_ap = inp("onesb", [1, 128], BF16)
    b3winT_ap = inp("b3winT_c", [128, FL], F32)
    b3woutT_ap = inp("b3woutT_c", [128, FL], F32)
    w3bT_ap = inp("w3bT_c", [N, FL], BF16)
    w3gateT_ap = inp("w3gateT_c", [N, FL], BF16)
    b3b_ap = inp("b3b_c", [128, NQ], F32)
    b3gate_ap = inp("b3gate_c", [128, NQ], F32)
    zt_ap = inp("ztk", [128, BL], BF16)
    eye_ap = inp("eyeb", [128, 128], BF16)
    out_ap = nc.dram_tensor("out", [Z + 1, BL], F32,
                            kind="ExternalOutput").ap()

    with tile.TileContext(nc) as tc:
        with tc.tile_pool(name="persist", bufs=1) as pp, \
             tc.tile_pool(name="strm_sy", bufs=8) as sp_sy, \
             tc.tile_pool(name="strm_sc", bufs=8) as sp_sc, \
             tc.tile_pool(name="strm_gp", bufs=4) as sp_gp, \
             tc.tile_pool(name="work", bufs=3) as wp, \
             tc.tile_pool(name="ccsb", bufs=2) as cp, \
             tc.tile_pool(name="ps_misc", bufs=1, space="PSUM") as ps_misc, \
             tc.tile_pool(name="ps_mv", bufs=1, space="PSUM") as ps_mv, \
             tc.tile_pool(name="ps_h", bufs=2, space="PSUM") as ps_h, \
             tc.tile_pool(name="ps_dz", bufs=1, space="PSUM") as ps_dz, \
             tc.tile_pool(name="ps_tr", bufs=1, space="PSUM") as ps_tr, \
             tc.tile_pool(name="dram", bufs=1, space="DRAM") as dp:

            # ---- small loads (SWDGE queue; issue before the big stream) --
            t_bc = pp.tile([128, 1], F32, tag="tbc")
            nc.gpsimd.dma_start(t_bc[:], t_ap.broadcast_to([128, 1]))
            w1_sb = pp.tile([128, 8], F32, tag="w1")
            b1_sb = pp.tile([128, 8], F32, tag="b1")
            b2_sb = pp.tile([128, 8], F32, tag="b2")
            w2t_sb = pp.tile([128, 2048], BF16, tag="w2t")
            nc.gpsimd.dma_start(w1_sb[:], w1_ap[:])
            nc.gpsimd.dma_start(b1_sb[:], b1_ap[:])
            nc.gpsimd.dma_start(b2_sb[:], b2_ap[:])
            nc.gpsimd.dma_start(w2t_sb[:], w2t_ap[:])
            zt_sb = pp.tile([128, BL], BF16, tag="zt")
            nc.gpsimd.dma_start(zt_sb[:], zt_ap[:])
            eye_sb = pp.tile([128, 128], BF16, tag="eye")
            nc.gpsimd.dma_start(eye_sb[:], eye_ap[:])
            b3winT_sb = pp.tile([128, FL], F32, tag="b3winT")
            b3woutT_sb = pp.tile([128, FL], F32, tag="b3woutT")
            nc.gpsimd.dma_start(b3winT_sb[:], b3winT_ap[:])
            nc.gpsimd.dma_start(b3woutT_sb[:], b3woutT_ap[:])
            w3h_sb = []
            for m, ap in (("b", w3bT_ap), ("gate", w3gateT_ap)):
                hb = pp.tile([128, 2 * FL], BF16, tag=f"w3{m}T")
                nc.gpsimd.dma_start(
                    hb[:], ap.rearrange("(nb p) fl -> p nb fl", p=128))
                w3h_sb.append(hb)
            b3b_sb = pp.tile([128, NQ], F32, tag="b3b")
            b3gate_sb = pp.tile([128, NQ], F32, tag="b3gate")
            nc.gpsimd.dma_start(b3b_sb[:], b3b_ap[:])
            nc.gpsimd.dma_start(b3gate_sb[:], b3gate_ap[:])

            # ---- parameter nets (tiny) ----------------------------------
            h0pre = pp.tile([128, 8], F32, tag="h0pre")
            nc.vector.tensor_scalar_mul(h0pre[:], w1_sb[:], t_bc[:, 0:1])
            nc.vector.tensor_add(h0pre[:], h0pre[:], b1_sb[:])
            h0_sb = pp.tile([128, 8], BF16, tag="h0")
            nc.scalar.activation(h0_sb[:], h0pre[:],
                                 mybir.ActivationFunctionType.Tanh)
            ps_h1 = ps_misc.tile([128, 8], F32, tag="misc")
            for k4 in range(4):
                for nb in range(2):
                    c = k4 * 2 + nb
                    for mb in range(2):
                        lhs = w2t_sb[:, k4 * 512 + mb * 256 + nb * 128:
                                     k4 * 512 + mb * 256 + nb * 128 + 128]
                        nc.tensor.matmul(ps_h1[:, c:c + 1], lhs,
                                         h0_sb[:, k4 * 2 + mb:k4 * 2 + mb + 1],
                                         start=(mb == 0), stop=(mb == 1))
            h1pre = pp.tile([128, 8], F32, tag="h1pre")
            h1_sb = pp.tile([128, 8], BF16, tag="h1")
            nc.vector.tensor_add(h1pre[:], ps_h1[:], b2_sb[:])
            nc.scalar.activation(h1_sb[:], h1pre[:],
                                 mybir.ActivationFunctionType.Tanh)

            # ---- h1 replicated across partitions for the DVE matvec ----
            ones_sb = pp.tile([1, 128], BF16, tag="ones")
            nc.gpsimd.dma_start(ones_sb[:], ones_ap[:])
            h1rep = []
            for net in range(2):
                h1row = pp.tile([1, N], BF16, tag=f"h1row{net}")
                for nb in range(2):
                    h1T = ps_misc.tile([1, 128], BF16, tag="misc",
                                       name="h1T")
                    nc.tensor.transpose(
                        h1T[:], h1_sb[:, net * 2 + nb:net * 2 + nb + 1],
                        eye_sb[:])
                    nc.scalar.activation(
                        h1row[0:1, nb * 128:(nb + 1) * 128],
                        h1T[:], mybir.ActivationFunctionType.Identity)
                ps_rep = ps_misc.tile([128, N], F32, tag="misc", name="psrep")
                nc.tensor.matmul(ps_rep[:], ones_sb[:], h1row[:],
                                 start=True, stop=True)
                hr = pp.tile([128, N], BF16, tag=f"h1rep{net}")
                nc.vector.tensor_copy(hr[:], ps_rep[:])
                h1rep.append(hr)

            # ---- heads: b and gate (local f, [128 f, 2] psum cols) ------
            b_loc = pp.tile([128, NQ], F32, tag="bloc")
            gate_loc = pp.tile([128, NQ], F32, tag="gateloc")
            gpre = pp.tile([128, NQ], F32, tag="gpre")
            for hb, dst, net in ((w3h_sb[0], b_loc, 2), (w3h_sb[1], gpre, 3)):
                ph = ps_misc.tile([128, NQ], F32, tag="misc", name="phd")
                for j in range(NQ):
                    for nb in range(2):
                        nc.tensor.matmul(
                            ph[:, j:j + 1],
                            hb[:, nb * FL + j * 128:nb * FL + (j + 1) * 128],
                            h1_sb[:, net * 2 + nb:net * 2 + nb + 1],
                            start=(nb == 0), stop=(nb == 1))
                bias = b3b_sb if net == 2 else b3gate_sb
                nc.vector.tensor_add(dst[:], ph[:], bias[:])
            nc.scalar.activation(gate_loc[:], gpre[:],
                                 mybir.ActivationFunctionType.Sigmoid)

            # ---- phase A: PE matvec + per-chunk AllGather ---------------
            cc_in = [dp.tile([128, CCW], BF16, tag=f"ccin{q}",
                             name=f"ccin{q}") for q in range(NQ)]
            cc_out = [dp.tile([n_cores, 128, CCW], BF16, tag=f"ccout{q}",
                              name=f"ccout{q}") for q in range(NQ)]
            ag_sb = []
            sg_f32 = pp.tile([128, NQ], F32, tag="sg")
            # engine stripe: 2x sync, 2x scalar, 1x gpsimd per 5 tiles —
            # ring pacing on each queue then tracks consumption order.
            stripe = [(nc.sync, sp_sy), (nc.scalar, sp_sc),
                      (nc.sync, sp_sy), (nc.scalar, sp_sc),
                      (nc.gpsimd, sp_gp)]
            for q in range(NQ):
                # chunk's stream DMA issue, in consumption order
                streamT = {}
                streamN = {}
                ti = 0
                for m in range(2):
                    for rc in range(nrc):
                        for nb in range(2):
                            eng, pool = stripe[ti % 5]
                            ti += 1
                            w3t = pool.tile([128, CHUNK_R], BF16, tag="w3s")
                            eng.dma_start(
                                w3t[:],
                                w3T_aps[m][q][nb * 128:(nb + 1) * 128,
                                              rc * CHUNK_R:(rc + 1) * CHUNK_R])
                            streamT[(m, rc, nb)] = w3t
                    for t in range(ndt):
                        eng, pool = stripe[ti % 5]
                        ti += 1
                        w3n = pool.tile([128, DCC * N], BF16, tag="w3s")
                        eng.dma_start(
                            w3n[:], w3N_aps[m][q][t * 128:(t + 1) * 128, :])
                        streamN[(m, t)] = w3n
                mv = []
                acc = []
                for m, net in ((0, 0), (1, 1)):
                    pw = ps_mv.tile([128, PE_F], F32, tag="mv", name=f"mv{m}")
                    for rc in range(nrc):
                        for j in range(cpr):
                            col = rc * cpr + j
                            for nb in range(2):
                                w3t = streamT[(m, rc, nb)]
                                nc.tensor.matmul(
                                    pw[:, col:col + 1],
                                    w3t[:, j * 128:(j + 1) * 128],
                                    h1_sb[:, net * 2 + nb:net * 2 + nb + 1],
                                    start=(nb == 0), stop=(nb == 1))
                    mv.append(pw)
                    # DVE part: natural-layout multiply by replicated h1,
                    # then reduce over n.
                    ad = wp.tile([128, DVE_F], F32, tag=f"acc{m}")
                    hr_bc = h1rep[net][:].unsqueeze(1).broadcast_to(
                        [128, DCC, N])
                    for t in range(ndt):
                        w3n = streamN[(m, t)]
                        prod = wp.tile([128, DCC * N], BF16, tag="prod")
                        nc.vector.tensor_mul(
                            prod[:].rearrange("p (c n) -> p c n", n=N),
                            w3n[:].rearrange("p (c n) -> p c n", n=N), hr_bc)
                        nc.vector.tensor_reduce(
                            ad[:, t * DCC:(t + 1) * DCC],
                            prod[:].rearrange("p (c n) -> p c n", n=N),
                            mybir.AxisListType.X, ADD)
                    acc.append(ad)
                cc_sb = cp.tile([128, CCW], BF16, tag="ccsb")
                # w_inT (+bias) -> cc cols [0, FQ)
                nc.vector.tensor_add(cc_sb[:, 0:PE_F], mv[0][:],
                                     b3winT_sb[:, q * FQ:q * FQ + PE_F])
                nc.vector.tensor_add(cc_sb[:, PE_F:FQ], acc[0][:],
                                     b3winT_sb[:, q * FQ + PE_F:(q + 1) * FQ])
                woutTb = wp.tile([128, FQ], BF16, tag="woutTb")
                nc.vector.tensor_add(woutTb[:, 0:PE_F], mv[1][:],
                                     b3woutT_sb[:, q * FQ:q * FQ + PE_F])
                nc.vector.tensor_add(woutTb[:, PE_F:FQ], acc[1][:],
                                     b3woutT_sb[:, q * FQ + PE_F:(q + 1) * FQ])
                tpsB = ps_misc.tile([128, 128], BF16, tag="misc", name="tpsB")
                nc.tensor.transpose(tpsB[:], woutTb[:], eye_sb[:])
                # w_outg_fz = w_out_fz * gate -> cc cols [FQ, 2FQ)
                nc.vector.tensor_scalar_mul(cc_sb[:, FQ:2 * FQ], tpsB[:],
                                            gate_loc[:, q:q + 1])
                tpsA = ps_misc.tile([128, 128], BF16, tag="misc", name="tpsA")
                nc.tensor.transpose(tpsA[:], cc_sb[:, 0:FQ], eye_sb[:])
                prod = wp.tile([128, 128], F32, tag="sgprod")
                nc.vector.tensor_mul(prod[:], tpsA[:], cc_sb[:, FQ:2 * FQ])
                nc.vector.tensor_reduce(sg_f32[:, q:q + 1], prod[:],
                                        mybir.AxisListType.X, ADD)
                nc.vector.tensor_copy(cc_sb[:, 2 * FQ:2 * FQ + 1],
                                      sg_f32[:, q:q + 1])
                nc.vector.tensor_copy(cc_sb[:, 2 * FQ + 1:2 * FQ + 2],
                                      b_loc[:, q:q + 1])
                nc.gpsimd.dma_start(cc_in[q][:], cc_sb[:])
                nc.gpsimd.collective_compute(
                    "AllGather", mybir.AluOpType.bypass,
                    replica_groups=[list(range(n_cores))],
                    ins=[cc_in[q].opt()], outs=[cc_out[q].opt()])
                ag = pp.tile([128, n_cores * CCW], BF16, tag=f"ag{q}")
                nc.gpsimd.dma_start(
                    ag[:], cc_out[q].rearrange("k p c -> p k c"))
                ag_sb.append(ag)

            # ---- phase B: B-sharded batch matmuls over full F -----------
            # f-blocks ordered q-major so all of AG chunk 0 is consumed
            # before anything waits on AG chunk 1.
            nj = BL // BC
            nfb = F // 128
            b_f32 = pp.tile([128, nfb], F32, tag="bf32")
            pdz = [ps_dz.tile([128, BC], F32, tag=f"pdz{j}", name=f"pdz{j}")
                   for j in range(nj)]
            ptr = [ps_tr.tile([1, BC], F32, tag=f"ptr{j}", name=f"ptr{j}")
                   for j in range(nj)]
            ag3d = [ag_sb[q].rearrange("p (k c) -> p k c", c=CCW)
                    for q in range(NQ)]
            for q in range(NQ):
                nc.vector.tensor_copy(
                    b_f32[:, q * n_cores:(q + 1) * n_cores],
                    ag3d[q][:, :, 2 * FQ + 1])
                for kk in range(n_cores):
                    i = q * n_cores + kk
                    ag = ag_sb[q]
                    lhT = ag[:, kk * CCW:kk * CCW + FQ]
                    lhD = ag[:, kk * CCW + FQ:kk * CCW + 2 * FQ]
                    sgc = ag[:, kk * CCW + 2 * FQ:kk * CCW + 2 * FQ + 1]
                    for j in range(nj):
                        ph = ps_h.tile([128, BC], F32, tag="ph")
                        nc.tensor.matmul(ph[:], lhT,
                                         zt_sb[:, j * BC:(j + 1) * BC],
                                         start=True, stop=True)
                        h_bf = wp.tile([128, BC], BF16, tag="hbf")
                        nc.scalar.activation(h_bf[:], ph[:],
                                             mybir.ActivationFunctionType.Tanh,
                                             bias=b_f32[:, i:i + 1])
                        h2_bf = wp.tile([128, BC], BF16, tag="h2bf")
                        nc.vector.tensor_mul(h2_bf[:], h_bf[:], h_bf[:])
                        nc.tensor.matmul(pdz[j][:], lhD, h_bf[:],
                                         start=(i == 0), stop=(i == nfb - 1))
                        nc.tensor.matmul(ptr[j][:], sgc, h2_bf[:],
                                         start=(i == 0), stop=(i == nfb - 1))
            # trace constant: cneg = -sum_f sg / F (issued after all h2
            # muls so the DVE FIFO never blocks on the second AllGather)
            s1 = pp.tile([128, NQ], F32, tag="s1")
            for q in range(NQ):
                nc.vector.tensor_reduce(s1[:, q:q + 1],
                                        ag3d[q][:, :, 2 * FQ],
                                        mybir.AxisListType.X, ADD)
            s1t = pp.tile([128, 1], F32, tag="s1t")
            nc.vector.tensor_reduce(s1t[:], s1[:], mybir.AxisListType.X, ADD)
            s128 = pp.tile([128, 1], F32, tag="s128")
            from concourse import bass_isa
            nc.gpsimd.partition_all_reduce(s128[:], s1t[:], 128,
                                           bass_isa.ReduceOp.add)
            cneg = pp.tile([1, 1], F32, tag="cneg")
            nc.scalar.mul(cneg[:], s128[0:1, 0:1], -1.0 / F)
            for j in range(nj):
                dz_sb = wp.tile([128, BC], F32, tag="dzsb")
                nc.scalar.mul(dz_sb[:], pdz[j][:], 1.0 / F)
                nc.sync.dma_start(out_ap[0:Z, j * BC:(j + 1) * BC], dz_sb[:])
                tr_sb = wp.tile([1, BC], F32, tag="trsb")
                nc.scalar.activation(tr_sb[:], ptr[j][:],
                                     mybir.ActivationFunctionType.Identity,
                                     bias=cneg[0:1, 0:1], scale=1.0 / F)
                nc.sync.dma_start(out_ap[Z:Z + 1, j * BC:(j + 1) * BC],
                                  tr_sb[:])

    nc.compile()
    return nc


def host_prep(t, z_and_logpz, W1, B1, W2, B2, W3_win, b3_win,
              W3_wout, b3_wout, W3_b, b3_b, W3_gate, b3_gate,
              n_cores=N_CORES):
    """Shard + lay out the numpy inputs into per-core in_maps."""
    def col8(x):  # [4, 256] -> [128, 8] with col = k*2 + nb
        return np.ascontiguousarray(
            np.asarray(x, np.float32).reshape(4, 2, 128).transpose(2, 0, 1)
            .reshape(128, 8))

    t_in = np.asarray(t, np.float32).reshape(1, 1)
    w1c = col8(np.asarray(W1, np.float32)[:, :, 0])
    b1c = col8(B1)
    b2c = col8(B2)
    w2tc = np.ascontiguousarray(
        np.asarray(W2, np.float32).transpose(0, 2, 1)
        .reshape(4, 2, 128, 256).transpose(2, 0, 1, 3).reshape(128, 2048)
    ).astype(BF)
    w3win_bf = np.asarray(W3_win, np.float32).astype(BF)
    w3wout_bf = np.asarray(W3_wout, np.float32).astype(BF)
    w3b_bf = np.asarray(W3_b, np.float32).astype(BF)
    w3gate_bf = np.asarray(W3_gate, np.float32).astype(BF)
    b3win = np.asarray(b3_win, np.float32)
    b3wout = np.asarray(b3_wout, np.float32)
    z = np.asarray(z_and_logpz, np.float32)[:, :Z]
    ztb = np.ascontiguousarray(z.T).astype(BF)   # [Z, B]
    eye = np.eye(128, dtype=np.float32).astype(BF)

    def pack_nat(x):  # [DVE_F*Z, N] -> [ndt*128, DCC*N], partition-contig
        nt = x.shape[0] // (DCC * 128)
        return np.ascontiguousarray(
            x.reshape(nt, DCC, 128, N).transpose(0, 2, 1, 3)
            .reshape(nt * 128, DCC * N))

    ones = np.ones((1, 128), dtype=np.float32).astype(BF)
    rows = FL * Z            # per-core W3 rows (32768)
    in_maps = []
    for k in range(n_cores):
        r0 = k * rows
        f0 = k * FL
        im = {
            "t": t_in, "w1c": w1c, "b1c": b1c, "b2c": b2c, "w2tc": w2tc,
            "onesb": ones,
            "b3winT_c": np.ascontiguousarray(
                b3win[r0:r0 + rows].reshape(FL, Z).T),
            "b3woutT_c": np.ascontiguousarray(
                b3wout[r0:r0 + rows].reshape(FL, Z).T),
            "w3bT_c": np.ascontiguousarray(w3b_bf[f0:f0 + FL].T),
            "w3gateT_c": np.ascontiguousarray(w3gate_bf[f0:f0 + FL].T),
            "b3b_c": np.ascontiguousarray(
                np.asarray(b3_b, np.float32)[f0:f0 + FL].reshape(NQ, 128).T),
            "b3gate_c": np.ascontiguousarray(
                np.asarray(b3_gate, np.float32)[f0:f0 + FL].reshape(NQ, 128).T),
            "ztk": np.ascontiguousarray(ztb[:, k * BL:(k + 1) * BL]),
            "eyeb": eye,
        }
        for q in range(NQ):
            rq0 = r0 + q * FQ * Z
            rpe = PE_F * Z
            rdv = DVE_F * Z
            im[f"w3winT_c{q}"] = np.ascontiguousarray(
                w3win_bf[rq0:rq0 + rpe].T)
            im[f"w3woutT_c{q}"] = np.ascontiguousarray(
                w3wout_bf[rq0:rq0 + rpe].T)
            im[f"w3winN_c{q}"] = pack_nat(w3win_bf[rq0 + rpe:rq0 + rpe + rdv])
            im[f"w3woutN_c{q}"] = pack_nat(
                w3wout_bf[rq0 + rpe:rq0 + rpe + rdv])
        in_maps.append(im)
    return in_maps


_NC_CACHE = {}


def kernel(**inputs) -> np.ndarray:
    _ensure_ntff_hook()
    from concourse import bass_utils

    key = "full"
    if key not in _NC_CACHE:
        _NC_CACHE[key] = build_module()
    nc = _NC_CACHE[key]

    in_maps = host_prep(**inputs)
    res = bass_utils.run_bass_kernel_spmd(nc, in_maps, list(range(N_CORES)))
    out = np.empty((B, Z + 1), np.float32)
    for k in range(N_CORES):
        out[k * BL:(k + 1) * BL, :] = res.results[k]["out"].T
    return out
